# revision 6
# baseline (speedup 1.0000x reference)
"""Trainium2 Bass kernel for nn_CharDistributionAnalyzer.

Per-row char histogram features over x:[B=262144, L=128] int32 tokens in [0, 40),
token 0 = padding. Output [B, 6] fp32:
  [unique/40, max_freq, min_freq(masked), letter_ratio, digit_ratio, special_ratio]

Strategy (pure data-parallel over 8 cores, 32768 rows each):
  - Tokens-transposed layout xt[128 tok, rows] bf16 per 2048-row super-block.
  - Bins 1..32 via DVE equality masks (4x tensor_scalar mode, ~680ns per
    [128,2048] tile); bins 33..39 + total via ACT Relu hinges
    H(t) = sum relu(x-t): counts are exact integer second differences, and
    total = m1 - H(1) with m1 = sum(x) from streaming xt itself through PE.
  - PE reduces all 42 streams over the token (partition) axis via stationary
    columns spread over 3 32-column groups with tile_position col-tiling so
    the three groups' matmuls run concurrently in the array.
  - fp32 transpose-back via perm matmul (H values exceed bf16 range), then
    small-tile decode + feature assembly.
"""

import numpy as np

import concourse.bass as bass
import concourse.bacc as bacc
import concourse.mybir as mybir
from concourse.tile import TileContext
from concourse.bass_utils import run_bass_kernel_spmd

N_CORES = 8
B_FULL = 262144
L = 128
V = 40
R_CORE = B_FULL // N_CORES  # 32768 rows per core

SB = 2048                  # rows per super-block
NBLK = SB // 128           # 16 token-transpose blocks per super-block
NBANK = SB // 512          # 4 psum bank-chunks per super-block

# streams: s=0..31 mask v=s+1; s=32..40 hinge t in {1,32,...,39}; s=41 m1(xt)
N_MASK = 32
HINGE_TS = [1, 32, 33, 34, 35, 36, 37, 38, 39]
N_HINGE = len(HINGE_TS)
N_STREAM = N_MASK + N_HINGE + 1  # 42
W_COLS = 32
S_LET, S_DIG = 14, 15      # within-group linear cols
PERM_D = 44                # 32 counts + 9 hinge + m1 + let + dig

AF = mybir.ActivationFunctionType
ALU = mybir.AluOpType
DT = mybir.dt
AX = mybir.AxisListType


def _stream_pos(s):
    g = s % 3
    slot = s // 3
    return g, slot


def build_bass(rows=R_CORE, gps_probe=True):
    """Build the per-core Bass module. `rows` must be a multiple of SB."""
    assert rows % SB == 0
    nsb = rows // SB

    nc = bacc.Bacc("TRN2")
    x = nc.dram_tensor("x", [rows, L], DT.int32, kind="ExternalInput")
    wall_d = nc.dram_tensor("wall", [128, N_STREAM * W_COLS], DT.bfloat16,
                            kind="ExternalInput")
    perm_d = nc.dram_tensor("perm", [96, PERM_D], DT.float32, kind="ExternalInput")
    out = nc.dram_tensor("out", [rows, 6], DT.float32, kind="ExternalOutput")

    with TileContext(nc) as tc:
        with (
            tc.tile_pool(name="const", bufs=1) as constp,
            tc.tile_pool(name="xraw", bufs=2) as xrawp,
            tc.tile_pool(name="xbf", bufs=2) as xbfp,
            tc.tile_pool(name="xt", bufs=2) as xtp,
            tc.tile_pool(name="mask", bufs=4) as maskp,
            tc.tile_pool(name="hinge", bufs=2) as hingep,
            tc.tile_pool(name="csb", bufs=2) as csbp,
            tc.tile_pool(name="cnt40", bufs=2) as cnt40p,
            tc.tile_pool(name="small", bufs=2) as smallp,
            tc.tile_pool(name="feat", bufs=2) as featp,
            tc.tile_pool(name="psum_c", bufs=2, space="PSUM") as psum_c,
            tc.tile_pool(name="psum_t", bufs=2, space="PSUM") as psum_t,
        ):
            # ---- constants ----
            w_all = constp.tile([128, N_STREAM * W_COLS], DT.bfloat16)
            nc.sync.dma_start(out=w_all[:], in_=wall_d[:, :])
            perm = constp.tile([96, PERM_D], DT.float32)
            nc.sync.dma_start(out=perm[:], in_=perm_d[:, :])
            hbias = constp.tile([128, N_HINGE], DT.float32)
            for k, t in enumerate(HINGE_TS):
                nc.vector.memset(hbias[:, k : k + 1], float(-t))

            for i in range(nsb):
                # ---- load + convert + transpose ----
                x_rows = x[i * SB : (i + 1) * SB, :].rearrange(
                    "(p j) l -> p j l", p=128
                )  # row = i*SB + p*NBLK + j
                xraw = xrawp.tile([128, NBLK, L], DT.int32)
                nc.sync.dma_start(out=xraw[:], in_=x_rows)

                xbf = xbfp.tile([128, NBLK, L], DT.bfloat16)
                # Relu == identity for x >= 0; keeps ACT on one table.
                nc.scalar.activation(out=xbf[:], in_=xraw[:], func=AF.Relu)

                xt = xtp.tile([128, NBLK, 128], DT.bfloat16)  # [tok, blk, rowpos]
                for j in range(NBLK):
                    nc.sync.dma_start_transpose(out=xt[:, j, :], in_=xbf[:, j, :])
                xt2d = xt[:].rearrange("t j r -> t (j r)")  # [128, SB]

                # ---- hinge tensors on ACT ----
                hing = hingep.tile([128, N_HINGE, SB], DT.bfloat16, tag="hinge")
                for k in range(N_HINGE):
                    nc.scalar.activation(
                        out=hing[:, k, :], in_=xt2d, func=AF.Relu,
                        bias=hbias[:, k : k + 1],
                    )

                # ---- streams -> PE accumulate (col-tiled over 3 groups) ----
                cntA = psum_c.tile([96, NBANK // 2, 512], DT.float32, tag="cnt")
                cntB = psum_c.tile([96, NBANK // 2, 512], DT.float32, tag="cnt")
                cnt_half = [cntA, cntB]

                def emit_stream(s, moving2d):
                    g, _slot = _stream_pos(s)
                    w_s = w_all[:, s * W_COLS : (s + 1) * W_COLS]
                    first = (s // 3) == 0
                    last = (s // 3) == (N_STREAM // 3 - 1)
                    for b in range(NBANK):
                        nc.tensor.matmul(
                            cnt_half[b // 2][32 * g : 32 * g + W_COLS, b % 2, :],
                            w_s,
                            moving2d[:, b * 512 : (b + 1) * 512],
                            start=first,
                            stop=last,
                            skip_group_check=True,
                            tile_position=(0, 32 * g),
                        )

                for s in range(N_MASK):
                    v = s + 1
                    mask = maskp.tile([128, SB], DT.bfloat16, tag="mask")
                    if gps_probe and i == 0 and v == 32:
                        nc.gpsimd.tensor_scalar(
                            out=mask[:], in0=xt2d, scalar1=float(v), scalar2=None,
                            op0=ALU.is_equal,
                        )
                    else:
                        nc.vector.tensor_scalar(
                            out=mask[:], in0=xt2d, scalar1=float(v), scalar2=None,
                            op0=ALU.is_equal,
                        )
                    emit_stream(s, mask[:])
                for k in range(N_HINGE):
                    emit_stream(N_MASK + k, hing[:, k, :])
                emit_stream(N_MASK + N_HINGE, xt2d)

                # ---- counts -> SBUF(fp32) -> transpose+permute to rows ----
                csb = csbp.tile([96, NBANK * 512], DT.float32)
                for h in range(2):
                    nc.scalar.activation(
                        out=csb[:, h * 1024 : (h + 1) * 1024],
                        in_=cnt_half[h][:].rearrange("p b f -> p (b f)"),
                        func=AF.Relu,
                    )

                tr = psum_t.tile([128, NBLK, 64], DT.float32)
                for j in range(NBLK):
                    nc.tensor.matmul(
                        tr[:, j, 0:PERM_D],
                        csb[:, j * 128 : (j + 1) * 128],
                        perm[:],
                        start=True,
                        stop=True,
                        skip_group_check=True,
                    )

                # tr cols: 0..31 counts v=1..32; 32..40 H(1),H(32..39);
                #          41 m1; 42 letters; 43 digits
                # hinge/linear cols -> SBUF (ops may read at most one PSUM input)
                hsb = smallp.tile([128, NBLK, 12], DT.float32, tag="hsb")
                nc.scalar.activation(out=hsb[:], in_=tr[:, :, 32:44], func=AF.Relu)
                H1 = hsb[:, :, 0]
                Hs = hsb[:, :, 1:9]    # H(32..39)
                m1 = hsb[:, :, 9]
                letc = hsb[:, :, 10]

                # ---- assemble full 39-bin count grid in bf16 ----
                cnt40 = cnt40p.tile([128, NBLK, 39], DT.bfloat16)
                nc.scalar.activation(
                    out=cnt40[:, :, 0:32], in_=tr[:, :, 0:32], func=AF.Relu
                )
                # c_t = H(t-1) - 2H(t) + H(t+1), t=33..38 ; c39 = H38 - 2*H39
                sum2 = smallp.tile([128, NBLK, 6], DT.float32, tag="sum2")
                nc.vector.tensor_tensor(
                    out=sum2[:], in0=Hs[:, :, 0:6], in1=Hs[:, :, 2:8], op=ALU.add
                )
                nc.vector.scalar_tensor_tensor(
                    out=cnt40[:, :, 32:38], in0=Hs[:, :, 1:7], scalar=-2.0,
                    in1=sum2[:], op0=ALU.mult, op1=ALU.add,
                )
                nc.vector.scalar_tensor_tensor(
                    out=cnt40[:, :, 38], in0=Hs[:, :, 7], scalar=-2.0,
                    in1=Hs[:, :, 6], op0=ALU.mult, op1=ALU.add,
                )

                # ---- linear features ----
                total = smallp.tile([128, NBLK], DT.float32, tag="total")
                nc.vector.tensor_tensor(out=total[:], in0=m1, in1=H1, op=ALU.subtract)
                # digits 27..36 = digcol(27..32) + c33..36 ; specials = c37..39
                dpart = smallp.tile([128, NBLK], DT.float32, tag="dpart")
                nc.vector.tensor_reduce(
                    out=dpart[:], in_=cnt40[:, :, 32:36], axis=AX.X, op=ALU.add
                )
                digc = smallp.tile([128, NBLK], DT.float32, tag="digc")
                nc.vector.tensor_tensor(
                    out=digc[:], in0=hsb[:, :, 11], in1=dpart[:], op=ALU.add
                )
                spec = smallp.tile([128, NBLK], DT.float32, tag="spec")
                nc.vector.tensor_reduce(
                    out=spec[:], in_=cnt40[:, :, 36:39], axis=AX.X, op=ALU.add
                )

                # ---- nonlinear features over the 39-bin grid ----
                pm = smallp.tile([128, NBLK, 39], DT.bfloat16, tag="pm")
                nc.vector.tensor_scalar(
                    out=pm[:], in0=cnt40[:], scalar1=0.5, scalar2=1024.0,
                    op0=ALU.is_lt, op1=ALU.mult,
                )  # 1024 where count == 0
                mmin = smallp.tile([128, NBLK, 39], DT.bfloat16, tag="mmin")
                nc.vector.tensor_tensor(
                    out=mmin[:], in0=cnt40[:], in1=pm[:], op=ALU.add
                )
                maxc = smallp.tile([128, NBLK], DT.float32, tag="maxc")
                nc.vector.tensor_reduce(out=maxc[:], in_=cnt40[:], axis=AX.X, op=ALU.max)
                minc = smallp.tile([128, NBLK], DT.float32, tag="minc")
                nc.vector.tensor_reduce(out=minc[:], in_=mmin[:], axis=AX.X, op=ALU.min)
                spos = smallp.tile([128, NBLK], DT.float32, tag="spos")
                nc.vector.tensor_reduce(out=spos[:], in_=pm[:], axis=AX.X, op=ALU.add)

                gate = smallp.tile([128, NBLK], DT.float32, tag="gate")
                nc.vector.tensor_scalar(
                    out=gate[:], in0=total[:], scalar1=0.5, scalar2=None, op0=ALU.is_gt
                )
                tc_ = smallp.tile([128, NBLK], DT.float32, tag="tc")
                nc.vector.tensor_scalar(
                    out=tc_[:], in0=total[:], scalar1=1.0, scalar2=None, op0=ALU.max
                )
                invt = smallp.tile([128, NBLK], DT.float32, tag="invt")
                nc.vector.reciprocal(out=invt[:], in_=tc_[:])

                feat = featp.tile([128, NBLK, 6], DT.float32)
                # unique = (39 - spos/1024) / 40
                nc.vector.tensor_scalar(
                    out=feat[:, :, 0], in0=spos[:], scalar1=-1.0 / 40960.0,
                    scalar2=39.0 / 40.0, op0=ALU.mult, op1=ALU.add,
                )
                nc.vector.tensor_tensor(
                    out=feat[:, :, 1], in0=maxc[:], in1=invt[:], op=ALU.mult
                )
                tmp = smallp.tile([128, NBLK], DT.float32, tag="tmp")
                nc.vector.tensor_tensor(
                    out=tmp[:], in0=minc[:], in1=invt[:], op=ALU.mult
                )
                nc.vector.tensor_tensor(
                    out=feat[:, :, 2], in0=tmp[:], in1=gate[:], op=ALU.mult
                )
                nc.vector.tensor_tensor(
                    out=feat[:, :, 3], in0=letc, in1=invt[:], op=ALU.mult
                )
                nc.vector.tensor_tensor(
                    out=feat[:, :, 4], in0=digc[:], in1=invt[:], op=ALU.mult
                )
                nc.vector.tensor_tensor(
                    out=feat[:, :, 5], in0=spec[:], in1=invt[:], op=ALU.mult
                )

                out_rows = out[i * SB : (i + 1) * SB, :].rearrange(
                    "(p j) f -> p j f", p=128
                )
                nc.sync.dma_start(out=out_rows, in_=feat[:])

    nc.compile()
    return nc


def build_wall():
    import ml_dtypes
    w = np.zeros((128, N_STREAM * W_COLS), np.float32)
    for s in range(N_STREAM):
        g, slot = _stream_pos(s)
        base = s * W_COLS
        w[:, base + slot] = 1.0
        if s < N_MASK:
            v = s + 1
            if 1 <= v <= 26:
                w[:, base + S_LET] = 1.0
            elif 27 <= v <= 36:
                w[:, base + S_DIG] = 1.0
    return w.astype(ml_dtypes.bfloat16)


def build_perm():
    p = np.zeros((96, PERM_D), np.float32)
    for d in range(PERM_D):
        if d < 41:
            s = d  # streams 0..40 (masks then hinges) in order
            g, slot = _stream_pos(s)
            p[32 * g + slot, d] = 1.0
        elif d == 41:
            g, slot = _stream_pos(41)
            p[32 * g + slot, d] = 1.0
        elif d == 42:
            for g in range(3):
                p[32 * g + S_LET, d] = 1.0
        elif d == 43:
            for g in range(3):
                p[32 * g + S_DIG, d] = 1.0
    return p


_NC_CACHE = {}


def _get_nc():
    if "nc" not in _NC_CACHE:
        _NC_CACHE["nc"] = build_bass()
    return _NC_CACHE["nc"]


def kernel(x: np.ndarray) -> np.ndarray:
    x = np.asarray(x, dtype=np.int32)
    assert x.shape == (B_FULL, L), x.shape
    nc = _get_nc()
    wall, perm = build_wall(), build_perm()
    in_maps = [
        {
            "x": np.ascontiguousarray(x[c * R_CORE : (c + 1) * R_CORE]),
            "wall": wall,
            "perm": perm,
        }
        for c in range(N_CORES)
    ]
    res = run_bass_kernel_spmd(nc, in_maps, core_ids=list(range(N_CORES)))
    return np.concatenate([res.results[c]["out"] for c in range(N_CORES)], axis=0)
